# revision 1
# baseline (speedup 1.0000x reference)
"""Trainium2 Bass kernel: single-head causal attention.

B=4, T=4096, E=512, H=64, fp32 in/out.

Sharding: 2 cores per batch sample. Each core computes partial softmax
(numerator and denominator) for ALL 4096 queries of its sample over HALF
the keys: core 2b takes even 128-key-strips, core 2b+1 odd strips. This
keeps the SPMD program structurally identical on every core and
perfectly load-balanced. The host combines partials:
out = (num0+num1)/(den0+den1).

To keep the program core-independent while each core selects different
key tokens, the host ROTATES every 256-token block by 128*rho (a half
swap, involutive). After rotation, each core's key strips are the first
128 tokens of every 256-block — a fixed slice pattern. Q columns are
then in rotated order: the host un-permutes output columns, and the two
causal mask tiles are built with the rotation baked in (the mask
pattern stays chunk-independent).

Device kernel per core (all matmul operands bf16, fp32 PSUM accumulate):
  - x^T resident in SBUF, (quarter, e-strip)-blocked so DMA runs are
    8KB/partition and matmul reads are contiguous.
  - QKV projections; K^T/V^T produced packed ([Wk|Wv] stationary ->
    PSUM rows 0:64 = K^T chunk, rows 64:128 = V^T chunk).
  - V^T -> V (natural [k,h]) via PE transpose + DVE cast.
  - Scores in S^T=[k,q] layout (K^T strip stationary, Q^T moving) so the
    softmax key-sum reduces over the PARTITION dim and comes free via a
    ones-column appended to V in the PV matmul.
  - exp on the scalar engine with fused 1/sqrt(H) scale; no max
    subtraction (scores bounded; fp32 exp cannot overflow here).
  - Causal mask applied multiplicatively after exp on the last two
    strips of each chunk.
"""

import functools

import numpy as np
import ml_dtypes

B, T, E, H = 4, 4096, 512, 64
NCORES = 8
NCHUNK = 8  # 512-query chunks per sample
CHUNK = T // NCHUNK  # 512
NSTRIP = 16  # local 128-key strips per core (half of T/128)
VSTRIDE = 80  # per-strip stride in the packed V tile

bf16 = ml_dtypes.bfloat16


@functools.lru_cache(maxsize=1)
def _build():
    import concourse.mybir as mybir
    from concourse import bacc
    from concourse.masks import make_identity
    import concourse.tile as tile

    dt_bf = mybir.dt.bfloat16
    dt_f32 = mybir.dt.float32

    nc = bacc.Bacc("TRN2", target_bir_lowering=False, num_devices=NCORES)

    # x^T, rotated, (quarter, e-strip)-blocked:
    # [4 quarters, 128, 4 e-strips, 1024 tokens]
    xt = nc.dram_tensor("xt", [4, 128, 4, T // 4], dt_bf, kind="ExternalInput")
    wq = nc.dram_tensor("wq", [128, 4 * 64], dt_bf, kind="ExternalInput")
    wkv = nc.dram_tensor("wkv", [128, 4 * 128], dt_bf, kind="ExternalInput")
    bias_q = nc.dram_tensor("bias_q", [64, 1], dt_f32, kind="ExternalInput")
    bias_kv = nc.dram_tensor("bias_kv", [128, 1], dt_f32, kind="ExternalInput")
    masks = nc.dram_tensor("masks", [128, 2 * CHUNK], dt_bf, kind="ExternalInput")
    out_d = nc.dram_tensor("out", [H + 1, T], dt_f32, kind="ExternalOutput")

    with tile.TileContext(nc) as tc:
        with (
            tc.tile_pool(name="const", bufs=1) as cpool,
            tc.tile_pool(name="xt_pool", bufs=1) as xpool,
            tc.tile_pool(name="q_pool", bufs=NCHUNK) as qpool,
            tc.tile_pool(name="kv_pool", bufs=4) as kvpool,
            tc.tile_pool(name="v_pool", bufs=1) as vpool,
            tc.tile_pool(name="p_pool", bufs=3) as ppool,
            tc.tile_pool(name="o_pool", bufs=2) as opool,
            tc.tile_pool(name="ps_proj", bufs=2, space="PSUM") as pspr_pool,
            tc.tile_pool(name="ps_s", bufs=2, space="PSUM") as pss_pool,
            tc.tile_pool(name="ps_o", bufs=2, space="PSUM") as pso_pool,
        ):
            # ---- input DMAs, upfront; issue split across both HWDGE
            # engines (Sync + Scalar) so issue latency doesn't serialize ----
            # xt_sb block (qd, es) occupies [:, (qd*4+es)*1024 : +1024]
            xt_sb = xpool.tile([128, 4 * T], dt_bf)
            wkv_sb = cpool.tile([128, 4 * 128], dt_bf)
            nc.sync.dma_start(wkv_sb, wkv.ap())
            wq_sb = cpool.tile([128, 4 * 64], dt_bf)
            nc.sync.dma_start(wq_sb, wq.ap())
            nc.sync.dma_start(
                xt_sb[:, 0 : T // 2],
                xt.ap()[0][:, 0:2, :].rearrange("p a t -> p (a t)"),
            )
            nc.sync.dma_start(
                xt_sb[:, T // 2 : T],
                xt.ap()[0][:, 2:4, :].rearrange("p a t -> p (a t)"),
            )
            bkv_sb = cpool.tile([128, 1], dt_f32)
            nc.sync.dma_start(bkv_sb, bias_kv.ap())
            bq_sb = cpool.tile([64, 1], dt_f32)
            nc.sync.dma_start(bq_sb, bias_q.ap())
            nc.sync.dma_start(
                xt_sb[:, T : 3 * T // 2],
                xt.ap()[1][:, 0:2, :].rearrange("p a t -> p (a t)"),
            )
            nc.sync.dma_start(
                xt_sb[:, 3 * T // 2 : 2 * T],
                xt.ap()[1][:, 2:4, :].rearrange("p a t -> p (a t)"),
            )
            masks_sb = cpool.tile([128, 2 * CHUNK], dt_bf)
            nc.sync.dma_start(masks_sb, masks.ap())
            nc.sync.dma_start(
                xt_sb[:, 2 * T : 5 * T // 2],
                xt.ap()[2][:, 0:2, :].rearrange("p a t -> p (a t)"),
            )
            nc.sync.dma_start(
                xt_sb[:, 5 * T // 2 : 3 * T],
                xt.ap()[2][:, 2:4, :].rearrange("p a t -> p (a t)"),
            )
            nc.sync.dma_start(
                xt_sb[:, 3 * T : 4 * T], xt.ap()[3].rearrange("p a t -> p (a t)")
            )
            ident = cpool.tile([128, 128], dt_bf)
            make_identity(nc, ident)

            # packed V (natural [k,h] layout + ones column for denominator)
            v_nat = vpool.tile([128, NSTRIP * VSTRIDE], dt_bf)
            v3 = v_nat.rearrange("p (s c) -> p s c", c=VSTRIDE)
            nc.vector.memset(v3[:, :, 64:65], 1.0)

            def xt_block(qd, es):
                off = (qd * 4 + es) * 1024
                return xt_sb[:, off : off + 1024]

            scale = 1.0 / float(np.sqrt(H))
            kv_tiles = []
            q_tiles = []

            def kv_proj(ckv):
                ps_kv = pspr_pool.tile([128, CHUNK], dt_f32, tag="proj")
                for es in range(4):
                    # keys: first 128 tokens of each 256-block
                    key_rhs = xt_block(ckv, es).rearrange(
                        "p (a two b) -> p a two b", two=2, b=128
                    )[:, :, 0, :]
                    nc.tensor.matmul(
                        ps_kv,
                        lhsT=wkv_sb[:, es * 128 : (es + 1) * 128],
                        rhs=key_rhs,
                        start=(es == 0),
                        stop=(es == 3),
                    )
                kv_sb = kvpool.tile([128, CHUNK], dt_bf, tag="kv")
                nc.vector.tensor_scalar_add(kv_sb, ps_kv, bkv_sb)
                kv_tiles.append(kv_sb)

            def v_transpose(ckv):
                # V^T (rows 64:128) -> natural V strips via PE transpose.
                # Deferred off the kv->q->scores->exp head chain: V is only
                # needed by PV, which trails exp.
                kv_sb = kv_tiles[ckv]
                for j in range(4):
                    s = 4 * ckv + j
                    ps_tr = pspr_pool.tile([128, 128], dt_bf, tag="proj")
                    nc.tensor.transpose(
                        ps_tr, kv_sb[:, j * 128 : (j + 1) * 128], ident
                    )
                    nc.vector.tensor_copy(
                        v_nat[:, s * VSTRIDE : s * VSTRIDE + 64],
                        ps_tr[:, 64:128],
                    )

            def q_proj(c):
                ps_q = pspr_pool.tile([64, CHUNK], dt_f32, tag="proj")
                for es in range(4):
                    nc.tensor.matmul(
                        ps_q,
                        lhsT=wq_sb[:, es * 64 : (es + 1) * 64],
                        rhs=xt_block(c // 2, es)[
                            :, (c % 2) * CHUNK : (c % 2) * CHUNK + CHUNK
                        ],
                        start=(es == 0),
                        stop=(es == 3),
                    )
                q_sb = qpool.tile([64, CHUNK], dt_bf, tag="q")
                nc.vector.tensor_scalar_add(q_sb, ps_q, bq_sb)
                q_tiles.append(q_sb)

            # projections run one chunk ahead of attention; V transposes
            # emitted just before the attention chunk that first needs them
            kv_proj(0)
            q_proj(0)
            for c in range(NCHUNK):
                if c + 1 < NCHUNK:
                    if (c + 1) % 2 == 0:
                        kv_proj((c + 1) // 2)
                    q_proj(c + 1)
                if c % 2 == 0:
                    v_transpose(c // 2)

                # ---- attention: chunk c attends to local strips 0..2c+1 ----
                ns = 2 * (c + 1)
                ps_o = pso_pool.tile([H + 1, CHUNK], dt_f32, tag="pso")
                for g0 in range(0, ns, 2):
                    g = min(2, ns - g0)
                    ps_s = pss_pool.tile([128, 2 * CHUNK], dt_f32, tag="pss")
                    for i in range(g):
                        l = g0 + i
                        nc.tensor.matmul(
                            ps_s[:, i * CHUNK : (i + 1) * CHUNK],
                            lhsT=kv_tiles[l // 4][
                                0:64, (l % 4) * 128 : (l % 4 + 1) * 128
                            ],
                            rhs=q_tiles[c],
                            start=True,
                            stop=True,
                        )
                    p_sb = ppool.tile([128, 2 * CHUNK], dt_bf, tag="p")
                    nc.scalar.activation(
                        p_sb[:, : g * CHUNK],
                        ps_s[:, : g * CHUNK],
                        mybir.ActivationFunctionType.Exp,
                        scale=scale,
                    )
                    # causal mask on the last two strips (l = 2c, 2c+1)
                    for i in range(g):
                        l = g0 + i
                        if l >= ns - 2:
                            j = l - (ns - 2)
                            nc.vector.tensor_mul(
                                p_sb[:, i * CHUNK : (i + 1) * CHUNK],
                                p_sb[:, i * CHUNK : (i + 1) * CHUNK],
                                masks_sb[:, j * CHUNK : (j + 1) * CHUNK],
                            )
                    for i in range(g):
                        l = g0 + i
                        nc.tensor.matmul(
                            ps_o,
                            lhsT=v_nat[:, l * VSTRIDE : l * VSTRIDE + 65],
                            rhs=p_sb[:, i * CHUNK : (i + 1) * CHUNK],
                            start=(l == 0),
                            stop=(l == ns - 1),
                        )

                o_sb = opool.tile([H + 1, CHUNK], dt_f32, tag="o")
                nc.vector.tensor_copy(o_sb, ps_o)
                nc.sync.dma_start(
                    out_d.ap()[:, c * CHUNK : (c + 1) * CHUNK], o_sb
                )

    nc.compile()
    return nc


def _perm(rho):
    """Rotated-order permutation: rotated position i holds original token
    perm[i]. Involutive (half swap within each 256-block)."""
    i = np.arange(T)
    return (i // 256) * 256 + ((i % 256) + 128 * rho) % 256


def _make_in_maps(x, Wq, bq, Wk, bk, Wv, bv):
    wq_pack = np.ascontiguousarray(
        Wq.reshape(4, 128, 64).transpose(1, 0, 2).reshape(128, 256)
    ).astype(bf16)
    wkv_pack = np.ascontiguousarray(
        np.concatenate([Wk.reshape(4, 128, 64), Wv.reshape(4, 128, 64)], axis=2)
        .transpose(1, 0, 2)
        .reshape(128, 512)
    ).astype(bf16)
    bias_q = np.ascontiguousarray(bq[:, None]).astype(np.float32)
    bias_kv = np.ascontiguousarray(np.concatenate([bk, bv])[:, None]).astype(
        np.float32
    )

    kk = np.arange(128)[:, None]
    in_maps = []
    for b in range(B):
        xt_b = np.ascontiguousarray(x[b].T).astype(bf16).reshape(4, 128, T)
        for rho in range(2):
            perm = _perm(rho)
            xt_rot = xt_b[:, :, perm]  # rotated token order
            xt_in = np.ascontiguousarray(
                xt_rot.reshape(4, 128, 4, T // 4).transpose(2, 1, 0, 3)
            )
            # masks: columns are in rotated order; v = original
            # within-chunk offset of rotated column jcol (chunk-indep.)
            v = perm[:CHUNK]
            m0 = (kk - v[None, :] <= -128 * rho).astype(bf16)
            m1 = (kk - v[None, :] <= -256 - 128 * rho).astype(bf16)
            masks_np = np.ascontiguousarray(np.concatenate([m0, m1], axis=1))
            in_maps.append(
                {
                    "xt": xt_in,
                    "wq": wq_pack,
                    "wkv": wkv_pack,
                    "bias_q": bias_q,
                    "bias_kv": bias_kv,
                    "masks": masks_np,
                }
            )
    return in_maps


def _combine(results):
    out = np.empty((B, T, H), np.float32)
    p1 = _perm(1)
    for b in range(B):
        a0 = results[2 * b]["out"].astype(np.float64)
        a1 = results[2 * b + 1]["out"].astype(np.float64)
        a1 = a1[:, p1]  # un-rotate core-1 columns (involutive perm)
        num = a0[:H] + a1[:H]
        den = a0[H] + a1[H]
        out[b] = (num / den).T.astype(np.float32)
    return out


def _run(trace=False, **inputs):
    from concourse import bass_utils

    nc = _build()
    in_maps = _make_in_maps(
        np.asarray(inputs["x"], np.float32),
        np.asarray(inputs["Wq"], np.float32),
        np.asarray(inputs["bq"], np.float32),
        np.asarray(inputs["Wk"], np.float32),
        np.asarray(inputs["bk"], np.float32),
        np.asarray(inputs["Wv"], np.float32),
        np.asarray(inputs["bv"], np.float32),
    )
    res = bass_utils.run_bass_kernel_spmd(
        nc, in_maps, list(range(NCORES)), trace=trace
    )
    return _combine(res.results), res.exec_time_ns


def kernel(**inputs):
    out, _ = _run(trace=False, **inputs)
    return out



# revision 2
# speedup vs baseline: 1.0143x; 1.0143x over previous
"""Trainium2 Bass kernel: single-head causal attention.

B=4, T=4096, E=512, H=64, fp32 in/out.

Sharding: 2 cores per batch sample. Each core computes partial softmax
(numerator and denominator) for ALL 4096 queries of its sample over HALF
the keys: core 2b takes even 128-key-strips, core 2b+1 odd strips. This
keeps the SPMD program structurally identical on every core and
perfectly load-balanced. The host combines partials:
out = (num0+num1)/(den0+den1).

To keep the program core-independent while each core selects different
key tokens, the host ROTATES every 256-token block by 128*rho (a half
swap, involutive). After rotation, each core's key strips are the first
128 tokens of every 256-block - a fixed slice pattern. Q columns are
then in rotated order: the host un-permutes output columns, and the
causal mask tiles are built with the rotation baked in (the mask
pattern stays chunk-independent).

Device kernel per core (all matmul operands bf16, fp32 PSUM accumulate):
  - x^T resident in SBUF, (quarter, e-strip)-blocked.
  - QKV projections; K^T/V^T produced packed ([Wk|Wv] stationary ->
    PSUM rows 0:64 = K^T chunk, rows 64:128 = V^T chunk); PSUM->SBUF
    cast on the scalar engine (Identity act with bias).
  - Q projection duplicated to both partition halves ([Wq|Wq]
    stationary) so score matmuls can be ROW-TILED: the two strips of a
    pair run CONCURRENTLY in the PE array (strip A in array rows 0:64,
    strip B in rows 64:128, via operand base partitions). Odd strips'
    K^T is copied to partitions 64:128 by an SBUF->SBUF DMA.
  - V^T -> V (natural [k,h]) via PE transpose (4 per kv chunk into one
    bf16 PSUM tile) + one strided DVE copy per chunk.
  - Scores in S^T=[k,q] layout so the softmax key-sum reduces over the
    PARTITION dim and comes free via a ones-column appended to V in the
    PV matmul.
  - exp on the scalar engine with fused 1/sqrt(H) scale; no max
    subtraction (scores bounded; fp32 exp cannot overflow here).
  - Causal structure: for the diagonal pair of each chunk the
    fully-masked half of the odd strip is skipped entirely (scores,
    exp, and PV restricted to query cols 256:512), and the exp call
    covers only the needed [256:1024] window of the pair tile.
  - Causal mask applied multiplicatively after exp on the GPSIMD engine
    (256-col windows per diagonal strip).
"""

import functools

import numpy as np
import ml_dtypes

B, T, E, H = 4, 4096, 512, 64
NCORES = 8
NCHUNK = 8  # 512-query chunks per sample
CHUNK = T // NCHUNK  # 512
NSTRIP = 16  # local 128-key strips per core (half of T/128)
VSTRIDE = 80  # per-strip stride in the packed V tile

bf16 = ml_dtypes.bfloat16


@functools.lru_cache(maxsize=1)
def _build():
    import concourse.mybir as mybir
    from concourse import bacc
    from concourse.masks import make_identity
    import concourse.tile as tile

    dt_bf = mybir.dt.bfloat16
    dt_f32 = mybir.dt.float32

    nc = bacc.Bacc("TRN2", target_bir_lowering=False, num_devices=NCORES)

    # x^T, rotated, (quarter, e-strip)-blocked:
    # [4 quarters, 128, 4 e-strips, 1024 tokens]
    xt = nc.dram_tensor("xt", [4, 128, 4, T // 4], dt_bf, kind="ExternalInput")
    # [Wq | Wq] per e-strip (duplicated for row-tiled scores)
    wq = nc.dram_tensor("wq", [128, 4 * 128], dt_bf, kind="ExternalInput")
    wkv = nc.dram_tensor("wkv", [128, 4 * 128], dt_bf, kind="ExternalInput")
    bias_q = nc.dram_tensor("bias_q", [128, 1], dt_f32, kind="ExternalInput")
    bias_kv = nc.dram_tensor("bias_kv", [128, 1], dt_f32, kind="ExternalInput")
    masks = nc.dram_tensor("masks", [128, CHUNK], dt_bf, kind="ExternalInput")
    out_d = nc.dram_tensor("out", [H + 1, T], dt_f32, kind="ExternalOutput")

    AF = mybir.ActivationFunctionType

    with tile.TileContext(nc) as tc:
        with (
            tc.tile_pool(name="const", bufs=1) as cpool,
            tc.tile_pool(name="xt_pool", bufs=1) as xpool,
            tc.tile_pool(name="q_pool", bufs=NCHUNK) as qpool,
            tc.tile_pool(name="kv_pool", bufs=4) as kvpool,
            tc.tile_pool(name="khi_pool", bufs=4) as khipool,
            tc.tile_pool(name="v_pool", bufs=1) as vpool,
            tc.tile_pool(name="p_pool", bufs=3) as ppool,
            tc.tile_pool(name="o_pool", bufs=2) as opool,
            tc.tile_pool(name="ps_proj", bufs=2, space="PSUM") as pspr_pool,
            tc.tile_pool(name="ps_s", bufs=2, space="PSUM") as pss_pool,
            tc.tile_pool(name="ps_o", bufs=2, space="PSUM") as pso_pool,
        ):
            # ---- input DMAs. Weights/biases/masks on the scalar HWDGE
            # queue; the bulk x^T stream on the sync queue, first quarter
            # split per e-strip so the first kv matmul can start ASAP ----
            xt_sb = xpool.tile([128, 4 * T], dt_bf)
            wkv_sb = cpool.tile([128, 4 * 128], dt_bf)
            nc.scalar.dma_start(wkv_sb, wkv.ap())
            wq_sb = cpool.tile([128, 4 * 128], dt_bf)
            nc.scalar.dma_start(wq_sb, wq.ap())
            bkv_sb = cpool.tile([128, 1], dt_f32)
            nc.scalar.dma_start(bkv_sb, bias_kv.ap())
            bq_sb = cpool.tile([128, 1], dt_f32)
            nc.scalar.dma_start(bq_sb, bias_q.ap())
            masks_sb = cpool.tile([128, CHUNK], dt_bf)
            nc.scalar.dma_start(masks_sb, masks.ap())

            # xt_sb block (qd, es) occupies [:, (qd*4+es)*1024 : +1024]
            for es in range(4):
                nc.sync.dma_start(
                    xt_sb[:, es * 1024 : (es + 1) * 1024], xt.ap()[0][:, es, :]
                )
            for qd in range(1, 4):
                nc.sync.dma_start(
                    xt_sb[:, qd * T : qd * T + T // 2],
                    xt.ap()[qd][:, 0:2, :].rearrange("p a t -> p (a t)"),
                )
                nc.sync.dma_start(
                    xt_sb[:, qd * T + T // 2 : (qd + 1) * T],
                    xt.ap()[qd][:, 2:4, :].rearrange("p a t -> p (a t)"),
                )
            ident = cpool.tile([128, 128], dt_bf)
            make_identity(nc, ident)

            # packed V (natural [k,h] layout + ones column for denominator)
            v_nat = vpool.tile([128, NSTRIP * VSTRIDE], dt_bf)
            v3 = v_nat.rearrange("p (s c) -> p s c", c=VSTRIDE)
            nc.vector.memset(v3[:, :, 64:65], 1.0)

            def xt_block(qd, es):
                off = (qd * 4 + es) * 1024
                return xt_sb[:, off : off + 1024]

            scale = 1.0 / float(np.sqrt(H))
            kv_tiles = []
            khi_tiles = []
            q_tiles = []

            def kv_proj(ckv):
                ps_kv = pspr_pool.tile([128, CHUNK], dt_f32, tag="proj")
                for es in range(4):
                    # keys: first 128 tokens of each 256-block
                    key_rhs = xt_block(ckv, es).rearrange(
                        "p (a two b) -> p a two b", two=2, b=128
                    )[:, :, 0, :]
                    nc.tensor.matmul(
                        ps_kv,
                        lhsT=wkv_sb[:, es * 128 : (es + 1) * 128],
                        rhs=key_rhs,
                        start=(es == 0),
                        stop=(es == 3),
                    )
                kv_sb = kvpool.tile([128, CHUNK], dt_bf, tag="kv")
                nc.scalar.activation(
                    kv_sb, ps_kv, AF.Identity, bias=bkv_sb, scale=1.0
                )
                kv_tiles.append(kv_sb)
                # odd strips' K^T copied to partitions 64:128 for the
                # row-tiled scores (SBUF->SBUF DMA, partition-crossing)
                khi_sb = khipool.tile([128, 2 * 128], dt_bf, tag="khi")
                src = kv_sb[0:64, :].rearrange("p (s c) -> p s c", c=128)
                dst = khi_sb[64:128, :].rearrange("p (s c) -> p s c", c=128)
                nc.sync.dma_start(dst[:, 0, :], src[:, 1, :])
                nc.sync.dma_start(dst[:, 1, :], src[:, 3, :])
                khi_tiles.append(khi_sb)

            def v_transpose(ckv):
                # V^T (rows 64:128) -> natural V strips via PE transpose.
                # 4 transposes into one bf16 PSUM tile, then ONE strided
                # DVE copy into the packed V tile.
                kv_sb = kv_tiles[ckv]
                ps_tr = pspr_pool.tile([128, CHUNK], dt_bf, tag="proj")
                for j in range(4):
                    nc.tensor.transpose(
                        ps_tr[:, j * 128 : (j + 1) * 128],
                        kv_sb[:, j * 128 : (j + 1) * 128],
                        ident,
                    )
                src = ps_tr.rearrange("p (s c) -> p s c", c=128)[:, :, 64:128]
                nc.vector.tensor_copy(v3[:, 4 * ckv : 4 * ckv + 4, 0:64], src)

            def q_proj(c):
                ps_q = pspr_pool.tile([128, CHUNK], dt_f32, tag="proj")
                for es in range(4):
                    nc.tensor.matmul(
                        ps_q,
                        lhsT=wq_sb[:, es * 128 : (es + 1) * 128],
                        rhs=xt_block(c // 2, es)[
                            :, (c % 2) * CHUNK : (c % 2) * CHUNK + CHUNK
                        ],
                        start=(es == 0),
                        stop=(es == 3),
                    )
                q_sb = qpool.tile([128, CHUNK], dt_bf, tag="q")
                nc.scalar.activation(
                    q_sb, ps_q, AF.Identity, bias=bq_sb, scale=1.0
                )
                q_tiles.append(q_sb)

            def k_lo(l):
                # even strip l: K^T at partitions 0:64
                return kv_tiles[l // 4][0:64, (l % 4) * 128 : (l % 4 + 1) * 128]

            def k_hi(l):
                # odd strip l: K^T at partitions 64:128
                j = (l % 4) // 2
                return khi_tiles[l // 4][64:128, j * 128 : (j + 1) * 128]

            # projections run one chunk ahead of attention
            kv_proj(0)
            q_proj(0)
            for c in range(NCHUNK):
                if c + 1 < NCHUNK:
                    if (c + 1) % 2 == 0:
                        kv_proj((c + 1) // 2)
                    q_proj(c + 1)
                if c % 2 == 0:
                    v_transpose(c // 2)

                # ---- attention: chunk c attends to local strips 0..2c+1,
                # processed as row-tiled pairs (2j, 2j+1) ----
                ns = 2 * (c + 1)
                q_sb = q_tiles[c]
                ps_o = pso_pool.tile([H + 1, CHUNK], dt_f32, tag="pso")
                for j in range(c + 1):
                    le, lo = 2 * j, 2 * j + 1
                    diag = j == c
                    ps_s = pss_pool.tile([128, 2 * CHUNK], dt_f32, tag="pss")
                    p_sb = ppool.tile([128, 2 * CHUNK], dt_bf, tag="p")
                    if not diag:
                        # bank A <- even strip (rows 0:64), bank B <- odd
                        nc.tensor.matmul(
                            ps_s[:, 0:CHUNK],
                            lhsT=k_lo(le),
                            rhs=q_sb[0:64, :],
                            start=True,
                            stop=True,
                        )
                        nc.tensor.matmul(
                            ps_s[:, CHUNK : 2 * CHUNK],
                            lhsT=k_hi(lo),
                            rhs=q_sb[64:128, :],
                            start=True,
                            stop=True,
                        )
                        nc.scalar.activation(
                            p_sb, ps_s, AF.Exp, scale=scale
                        )
                    else:
                        # diagonal pair: odd strip only needs query cols
                        # 256:512 (rest fully masked); layout [odd | even]
                        # so one exp covers the contiguous [256:1024] window
                        nc.tensor.matmul(
                            ps_s[:, 256:CHUNK],
                            lhsT=k_hi(lo),
                            rhs=q_sb[64:128, 256:CHUNK],
                            start=True,
                            stop=True,
                        )
                        nc.tensor.matmul(
                            ps_s[:, CHUNK : 2 * CHUNK],
                            lhsT=k_lo(le),
                            rhs=q_sb[0:64, :],
                            start=True,
                            stop=True,
                        )
                        nc.scalar.activation(
                            p_sb[:, 256 : 2 * CHUNK],
                            ps_s[:, 256 : 2 * CHUNK],
                            AF.Exp,
                            scale=scale,
                        )
                        # causal masks (GPSIMD; SBUF-only operands):
                        # odd strip window cols 256:512, even strip cols 0:256
                        nc.gpsimd.tensor_mul(
                            p_sb[:, 256:CHUNK],
                            p_sb[:, 256:CHUNK],
                            masks_sb[:, 0:256],
                        )
                        nc.gpsimd.tensor_mul(
                            p_sb[:, CHUNK : CHUNK + 256],
                            p_sb[:, CHUNK : CHUNK + 256],
                            masks_sb[:, 256:512],
                        )

                    # ---- PV accumulation (ascending strip order) ----
                    if not diag:
                        nc.tensor.matmul(
                            ps_o,
                            lhsT=v_nat[:, le * VSTRIDE : le * VSTRIDE + 65],
                            rhs=p_sb[:, 0:CHUNK],
                            start=(le == 0),
                            stop=False,
                        )
                        nc.tensor.matmul(
                            ps_o,
                            lhsT=v_nat[:, lo * VSTRIDE : lo * VSTRIDE + 65],
                            rhs=p_sb[:, CHUNK : 2 * CHUNK],
                            start=False,
                            stop=(lo == ns - 1),
                        )
                    else:
                        nc.tensor.matmul(
                            ps_o,
                            lhsT=v_nat[:, le * VSTRIDE : le * VSTRIDE + 65],
                            rhs=p_sb[:, CHUNK : 2 * CHUNK],
                            start=(le == 0),
                            stop=False,
                        )
                        nc.tensor.matmul(
                            ps_o[:, 256:CHUNK],
                            lhsT=v_nat[:, lo * VSTRIDE : lo * VSTRIDE + 65],
                            rhs=p_sb[:, 256:CHUNK],
                            start=False,
                            stop=(lo == ns - 1),
                            skip_group_check=True,
                        )

                o_sb = opool.tile([H + 1, CHUNK], dt_f32, tag="o")
                nc.vector.tensor_copy(o_sb, ps_o)
                nc.sync.dma_start(
                    out_d.ap()[:, c * CHUNK : (c + 1) * CHUNK], o_sb
                )

    nc.compile()
    return nc


def _perm(rho):
    """Rotated-order permutation: rotated position i holds original token
    perm[i]. Involutive (half swap within each 256-block)."""
    i = np.arange(T)
    return (i // 256) * 256 + ((i % 256) + 128 * rho) % 256


def _make_in_maps(x, Wq, bq, Wk, bk, Wv, bv):
    wq_r = Wq.reshape(4, 128, 64)
    wq_pack = np.ascontiguousarray(
        np.concatenate([wq_r, wq_r], axis=2).transpose(1, 0, 2).reshape(128, 512)
    ).astype(bf16)
    wkv_pack = np.ascontiguousarray(
        np.concatenate([Wk.reshape(4, 128, 64), Wv.reshape(4, 128, 64)], axis=2)
        .transpose(1, 0, 2)
        .reshape(128, 512)
    ).astype(bf16)
    bias_q = np.ascontiguousarray(
        np.concatenate([bq, bq])[:, None]
    ).astype(np.float32)
    bias_kv = np.ascontiguousarray(np.concatenate([bk, bv])[:, None]).astype(
        np.float32
    )

    kk = np.arange(128)[:, None]
    in_maps = []
    for b in range(B):
        xt_b = np.ascontiguousarray(x[b].T).astype(bf16).reshape(4, 128, T)
        for rho in range(2):
            perm = _perm(rho)
            xt_rot = xt_b[:, :, perm]  # rotated token order
            xt_in = np.ascontiguousarray(
                xt_rot.reshape(4, 128, 4, T // 4).transpose(2, 1, 0, 3)
            )
            # masks: columns are in rotated order; v = original
            # within-chunk offset of rotated column jcol (chunk-indep.)
            v = perm[:CHUNK]
            m0 = (kk - v[None, :] <= -128 * rho).astype(bf16)
            m1 = (kk - v[None, :] <= -256 - 128 * rho).astype(bf16)
            # [odd-strip window cols 256:512 | even-strip cols 0:256]
            masks_np = np.ascontiguousarray(
                np.concatenate([m1[:, 256:512], m0[:, 0:256]], axis=1)
            )
            in_maps.append(
                {
                    "xt": xt_in,
                    "wq": wq_pack,
                    "wkv": wkv_pack,
                    "bias_q": bias_q,
                    "bias_kv": bias_kv,
                    "masks": masks_np,
                }
            )
    return in_maps


def _combine(results):
    out = np.empty((B, T, H), np.float32)
    p1 = _perm(1)
    for b in range(B):
        a0 = results[2 * b]["out"].astype(np.float64)
        a1 = results[2 * b + 1]["out"].astype(np.float64)
        a1 = a1[:, p1]  # un-rotate core-1 columns (involutive perm)
        num = a0[:H] + a1[:H]
        den = a0[H] + a1[H]
        out[b] = (num / den).T.astype(np.float32)
    return out


def _run(trace=False, **inputs):
    from concourse import bass_utils

    nc = _build()
    in_maps = _make_in_maps(
        np.asarray(inputs["x"], np.float32),
        np.asarray(inputs["Wq"], np.float32),
        np.asarray(inputs["bq"], np.float32),
        np.asarray(inputs["Wk"], np.float32),
        np.asarray(inputs["bk"], np.float32),
        np.asarray(inputs["Wv"], np.float32),
        np.asarray(inputs["bv"], np.float32),
    )
    res = bass_utils.run_bass_kernel_spmd(
        nc, in_maps, list(range(NCORES)), trace=trace
    )
    return _combine(res.results), res.exec_time_ns


def kernel(**inputs):
    out, _ = _run(trace=False, **inputs)
    return out


# revision 6
# speedup vs baseline: 1.0706x; 1.0554x over previous
"""Trainium2 Bass kernel: single-head causal attention.

B=4, T=4096, E=512, H=64, fp32 in/out.

Sharding: 2 cores per batch sample. Each core computes partial softmax
(numerator and denominator) for ALL 4096 queries of its sample over HALF
the keys: core 2b takes even 128-key-strips, core 2b+1 odd strips. The
host combines partials: out = (num0+num1)/(den0+den1).

Token rotation: the host rotates every 256-token block by 128*rho so
each core's keys are the first 128 tokens of every 256-block (identical
SPMD program on all cores). Host un-permutes output columns; the causal
masks carry the rotation.

Device kernel per core (bf16 matmul operands, fp32 PSUM accumulate):
  - ROW-TILED scores: the two strips of a pair run concurrently in the
    PE array (strip A rows 0:64, strip B rows 64:128 via operand base
    partitions). Q projection duplicated to both partition halves
    ([Wq|Wq] stationary); odd strips' K^T copied to partitions 64:128
    by SBUF->SBUF DMAs.
  - V^T -> V (natural [k,h]) via PE transpose (4 per kv chunk into one
    bf16 PSUM tile) + one strided DVE copy per chunk; a ones column is
    packed after each V strip so the PV matmul (M=65) also produces the
    softmax denominator (partition-dim reduction on the PE).
  - exp on the scalar engine with fused 1/sqrt(H) scale; no max
    subtraction (scores bounded; fp32 exp cannot overflow here).
  - Diagonal pair of each chunk: the fully-masked half of the odd strip
    is skipped (scores/exp/PV restricted to query cols 256:512); the
    exp covers the contiguous [256:1024] window of the pair tile.
  - Causal mask applied multiplicatively after exp on the vector
    engine (256-col windows per diagonal strip).
  - Emission is software-pipelined: scores/exp of pair j+1 are emitted
    before PV of pair j so the in-order PE never waits on exp.
  - PE warm-up matmuls run during the input DMA window so real matmuls
    start at the full 2.4 GHz clock (HAM un-throttled).
"""

import functools

import numpy as np
import ml_dtypes

B, T, E, H = 4, 4096, 512, 64
NCORES = 8
NCHUNK = 8  # 512-query chunks per sample
CHUNK = T // NCHUNK  # 512
NSTRIP = 16  # local 128-key strips per core (half of T/128)
VSTRIDE = 80  # per-strip stride in the packed V tile

bf16 = ml_dtypes.bfloat16


@functools.lru_cache(maxsize=1)
def _build():
    import concourse.mybir as mybir
    from concourse import bacc
    from concourse.masks import make_identity
    import concourse.tile as tile

    dt_bf = mybir.dt.bfloat16
    dt_f32 = mybir.dt.float32

    nc = bacc.Bacc("TRN2", target_bir_lowering=False, num_devices=NCORES)

    # x^T, rotated, (quarter, e-strip)-blocked:
    # [4 quarters, 128, 4 e-strips, 1024 tokens]
    xt = nc.dram_tensor("xt", [4, 128, 4, T // 4], dt_bf, kind="ExternalInput")
    # [Wq | Wq] per e-strip (duplicated for row-tiled scores)
    wq = nc.dram_tensor("wq", [128, 4 * 128], dt_bf, kind="ExternalInput")
    wkv = nc.dram_tensor("wkv", [128, 4 * 128], dt_bf, kind="ExternalInput")
    bias_q = nc.dram_tensor("bias_q", [128, 1], dt_f32, kind="ExternalInput")
    bias_kv = nc.dram_tensor("bias_kv", [128, 1], dt_f32, kind="ExternalInput")
    masks = nc.dram_tensor("masks", [128, CHUNK], dt_bf, kind="ExternalInput")
    out_d = nc.dram_tensor("out", [H + 1, T], dt_f32, kind="ExternalOutput")

    AF = mybir.ActivationFunctionType

    with tile.TileContext(nc) as tc:
        with (
            tc.tile_pool(name="const", bufs=1) as cpool,
            tc.tile_pool(name="xt_pool", bufs=1) as xpool,
            tc.tile_pool(name="q_pool", bufs=NCHUNK) as qpool,
            tc.tile_pool(name="kv_pool", bufs=4) as kvpool,
            tc.tile_pool(name="khi_pool", bufs=4) as khipool,
            tc.tile_pool(name="v_pool", bufs=1) as vpool,
            tc.tile_pool(name="p_pool", bufs=3) as ppool,
            tc.tile_pool(name="o_pool", bufs=2) as opool,
            tc.tile_pool(name="ps_proj", bufs=2, space="PSUM") as pspr_pool,
            tc.tile_pool(name="ps_s", bufs=2, space="PSUM") as pss_pool,
            tc.tile_pool(name="ps_o", bufs=2, space="PSUM") as pso_pool,
        ):
            # ---- input DMAs. wkv + first-quarter e-strips lead on the
            # sync queue (first kv matmul deps); weights/biases/masks and
            # later quarters stream behind on both queues ----
            xt_sb = xpool.tile([128, 4 * T], dt_bf)
            wkv_sb = cpool.tile([128, 4 * 128], dt_bf)
            nc.sync.dma_start(wkv_sb, wkv.ap())
            for es in range(4):
                nc.sync.dma_start(
                    xt_sb[:, es * 1024 : (es + 1) * 1024], xt.ap()[0][:, es, :]
                )
            wq_sb = cpool.tile([128, 4 * 128], dt_bf)
            nc.scalar.dma_start(wq_sb, wq.ap())
            bkv_sb = cpool.tile([128, 1], dt_f32)
            nc.scalar.dma_start(bkv_sb, bias_kv.ap())
            bq_sb = cpool.tile([128, 1], dt_f32)
            nc.scalar.dma_start(bq_sb, bias_q.ap())
            masks_sb = cpool.tile([128, CHUNK], dt_bf)
            nc.scalar.dma_start(masks_sb, masks.ap())
            for qd in range(1, 4):
                eng = nc.sync if qd == 1 else nc.scalar
                eng.dma_start(
                    xt_sb[:, qd * T : qd * T + T // 2],
                    xt.ap()[qd][:, 0:2, :].rearrange("p a t -> p (a t)"),
                )
                eng.dma_start(
                    xt_sb[:, qd * T + T // 2 : (qd + 1) * T],
                    xt.ap()[qd][:, 2:4, :].rearrange("p a t -> p (a t)"),
                )
            ident = cpool.tile([128, 128], dt_bf)
            make_identity(nc, ident)
            junk_sb = cpool.tile([128, CHUNK], dt_bf)
            nc.vector.memset(junk_sb, 0.0)

            # ---- PE warm-up: dummy matmuls fill the HAM activity window
            # while the first x^T DMAs land. Results never read ----
            for i in range(2):
                ps_w = pss_pool.tile([128, 2 * CHUNK], dt_f32, tag="pss")
                for r in range(2):
                    nc.tensor.matmul(
                        ps_w[:, r * CHUNK : (r + 1) * CHUNK],
                        lhsT=junk_sb[:, 0:128],
                        rhs=junk_sb,
                        start=True,
                        stop=True,
                        skip_group_check=True,
                    )

            # packed V (natural [k,h] layout + ones column for denominator)
            v_nat = vpool.tile([128, NSTRIP * VSTRIDE], dt_bf)
            v3 = v_nat.rearrange("p (s c) -> p s c", c=VSTRIDE)
            nc.vector.memset(v3[:, :, 64:65], 1.0)

            def xt_block(qd, es):
                off = (qd * 4 + es) * 1024
                return xt_sb[:, off : off + 1024]

            scale = 1.0 / float(np.sqrt(H))
            kv_tiles = []
            khi_tiles = []
            q_tiles = []

            def kv_proj(ckv):
                ps_kv = pspr_pool.tile([128, CHUNK], dt_f32, tag="proj")
                for es in range(4):
                    # keys: first 128 tokens of each 256-block
                    key_rhs = xt_block(ckv, es).rearrange(
                        "p (a two b) -> p a two b", two=2, b=128
                    )[:, :, 0, :]
                    nc.tensor.matmul(
                        ps_kv,
                        lhsT=wkv_sb[:, es * 128 : (es + 1) * 128],
                        rhs=key_rhs,
                        start=(es == 0),
                        stop=(es == 3),
                    )
                kv_sb = kvpool.tile([128, CHUNK], dt_bf, tag="kv")
                nc.vector.tensor_scalar_add(kv_sb, ps_kv, bkv_sb)
                kv_tiles.append(kv_sb)
                # odd strips' K^T copied to partitions 64:128 for the
                # row-tiled scores (SBUF->SBUF DMA, partition-crossing)
                khi_sb = khipool.tile([128, 2 * 128], dt_bf, tag="khi")
                src = kv_sb[0:64, :].rearrange("p (s c) -> p s c", c=128)
                dst = khi_sb[64:128, :].rearrange("p (s c) -> p s c", c=128)
                nc.sync.dma_start(dst[:, 0, :], src[:, 1, :])
                nc.sync.dma_start(dst[:, 1, :], src[:, 3, :])
                khi_tiles.append(khi_sb)

            def v_transpose(ckv):
                # V^T (rows 64:128) -> natural V strips via PE transpose.
                # 4 transposes into one bf16 PSUM tile, then ONE strided
                # DVE copy into the packed V tile.
                kv_sb = kv_tiles[ckv]
                ps_tr = pspr_pool.tile([128, CHUNK], dt_bf, tag="proj")
                for j in range(4):
                    nc.tensor.transpose(
                        ps_tr[:, j * 128 : (j + 1) * 128],
                        kv_sb[:, j * 128 : (j + 1) * 128],
                        ident,
                    )
                src = ps_tr.rearrange("p (s c) -> p s c", c=128)[:, :, 64:128]
                nc.vector.tensor_copy(v3[:, 4 * ckv : 4 * ckv + 4, 0:64], src)

            def q_proj(c):
                ps_q = pspr_pool.tile([128, CHUNK], dt_f32, tag="proj")
                for es in range(4):
                    nc.tensor.matmul(
                        ps_q,
                        lhsT=wq_sb[:, es * 128 : (es + 1) * 128],
                        rhs=xt_block(c // 2, es)[
                            :, (c % 2) * CHUNK : (c % 2) * CHUNK + CHUNK
                        ],
                        start=(es == 0),
                        stop=(es == 3),
                    )
                q_sb = qpool.tile([128, CHUNK], dt_bf, tag="q")
                nc.vector.tensor_scalar_add(q_sb, ps_q, bq_sb)
                q_tiles.append(q_sb)

            def k_lo(l):
                # even strip l: K^T at partitions 0:64
                return kv_tiles[l // 4][0:64, (l % 4) * 128 : (l % 4 + 1) * 128]

            def k_hi(l):
                # odd strip l: K^T at partitions 64:128
                j = (l % 4) // 2
                return khi_tiles[l // 4][64:128, j * 128 : (j + 1) * 128]

            # projections run one chunk ahead of attention
            kv_proj(0)
            q_proj(0)
            for c in range(NCHUNK):
                if c + 1 < NCHUNK:
                    if (c + 1) % 2 == 0:
                        kv_proj((c + 1) // 2)
                    q_proj(c + 1)
                if c % 2 == 0:
                    v_transpose(c // 2)

                ns = 2 * (c + 1)
                q_sb = q_tiles[c]
                ps_o = pso_pool.tile([H + 1, CHUNK], dt_f32, tag="pso")

                def scores_exp(j):
                    le, lo = 2 * j, 2 * j + 1
                    diag = j == c
                    ps_s = pss_pool.tile([128, 2 * CHUNK], dt_f32, tag="pss")
                    p_sb = ppool.tile([128, 2 * CHUNK], dt_bf, tag="p")
                    if not diag:
                        nc.tensor.matmul(
                            ps_s[:, 0:CHUNK],
                            lhsT=k_lo(le),
                            rhs=q_sb[0:64, :],
                            start=True,
                            stop=True,
                        )
                        nc.tensor.matmul(
                            ps_s[:, CHUNK : 2 * CHUNK],
                            lhsT=k_hi(lo),
                            rhs=q_sb[64:128, :],
                            start=True,
                            stop=True,
                        )
                        nc.scalar.activation(p_sb, ps_s, AF.Exp, scale=scale)
                    else:
                        # diagonal pair: odd strip only needs query cols
                        # 256:512; layout [odd | even] so one exp covers
                        # the contiguous [256:1024] window
                        nc.tensor.matmul(
                            ps_s[:, 256:CHUNK],
                            lhsT=k_hi(lo),
                            rhs=q_sb[64:128, 256:CHUNK],
                            start=True,
                            stop=True,
                        )
                        nc.tensor.matmul(
                            ps_s[:, CHUNK : 2 * CHUNK],
                            lhsT=k_lo(le),
                            rhs=q_sb[0:64, :],
                            start=True,
                            stop=True,
                        )
                        nc.scalar.activation(
                            p_sb[:, 256 : 2 * CHUNK],
                            ps_s[:, 256 : 2 * CHUNK],
                            AF.Exp,
                            scale=scale,
                        )
                        # causal masks: odd strip window cols 256:512,
                        # even strip cols 0:256
                        nc.vector.tensor_mul(
                            p_sb[:, 256:CHUNK],
                            p_sb[:, 256:CHUNK],
                            masks_sb[:, 0:256],
                        )
                        nc.vector.tensor_mul(
                            p_sb[:, CHUNK : CHUNK + 256],
                            p_sb[:, CHUNK : CHUNK + 256],
                            masks_sb[:, 256:512],
                        )
                    return p_sb

                def pv(j, p_sb):
                    le, lo = 2 * j, 2 * j + 1
                    diag = j == c
                    if not diag:
                        nc.tensor.matmul(
                            ps_o,
                            lhsT=v_nat[:, le * VSTRIDE : le * VSTRIDE + 65],
                            rhs=p_sb[:, 0:CHUNK],
                            start=(le == 0),
                            stop=False,
                        )
                        nc.tensor.matmul(
                            ps_o,
                            lhsT=v_nat[:, lo * VSTRIDE : lo * VSTRIDE + 65],
                            rhs=p_sb[:, CHUNK : 2 * CHUNK],
                            start=False,
                            stop=(lo == ns - 1),
                        )
                    else:
                        nc.tensor.matmul(
                            ps_o,
                            lhsT=v_nat[:, le * VSTRIDE : le * VSTRIDE + 65],
                            rhs=p_sb[:, CHUNK : 2 * CHUNK],
                            start=(le == 0),
                            stop=False,
                        )
                        nc.tensor.matmul(
                            ps_o[:, 256:CHUNK],
                            lhsT=v_nat[:, lo * VSTRIDE : lo * VSTRIDE + 65],
                            rhs=p_sb[:, 256:CHUNK],
                            start=False,
                            stop=(lo == ns - 1),
                            skip_group_check=True,
                        )

                # software pipeline: scores/exp one pair ahead of PV
                prev = scores_exp(0)
                for j in range(1, c + 1):
                    cur = scores_exp(j)
                    pv(j - 1, prev)
                    prev = cur
                pv(c, prev)

                o_sb = opool.tile([H + 1, CHUNK], dt_f32, tag="o")
                nc.vector.tensor_copy(o_sb, ps_o)
                nc.sync.dma_start(
                    out_d.ap()[:, c * CHUNK : (c + 1) * CHUNK], o_sb
                )

    nc.compile()
    return nc


def _perm(rho):
    """Rotated-order permutation: rotated position i holds original token
    perm[i]. Involutive (half swap within each 256-block)."""
    i = np.arange(T)
    return (i // 256) * 256 + ((i % 256) + 128 * rho) % 256


def _make_in_maps(x, Wq, bq, Wk, bk, Wv, bv):
    wq_r = Wq.reshape(4, 128, 64)
    wq_pack = np.ascontiguousarray(
        np.concatenate([wq_r, wq_r], axis=2).transpose(1, 0, 2).reshape(128, 512)
    ).astype(bf16)
    wkv_pack = np.ascontiguousarray(
        np.concatenate([Wk.reshape(4, 128, 64), Wv.reshape(4, 128, 64)], axis=2)
        .transpose(1, 0, 2)
        .reshape(128, 512)
    ).astype(bf16)
    bias_q = np.ascontiguousarray(
        np.concatenate([bq, bq])[:, None]
    ).astype(np.float32)
    bias_kv = np.ascontiguousarray(np.concatenate([bk, bv])[:, None]).astype(
        np.float32
    )

    kk = np.arange(128)[:, None]
    in_maps = []
    for b in range(B):
        xt_b = np.ascontiguousarray(x[b].T).astype(bf16).reshape(4, 128, T)
        for rho in range(2):
            perm = _perm(rho)
            xt_rot = xt_b[:, :, perm]  # rotated token order
            xt_in = np.ascontiguousarray(
                xt_rot.reshape(4, 128, 4, T // 4).transpose(2, 1, 0, 3)
            )
            v = perm[:CHUNK]
            m0 = (kk - v[None, :] <= -128 * rho).astype(bf16)
            m1 = (kk - v[None, :] <= -256 - 128 * rho).astype(bf16)
            # [odd-strip window cols 256:512 | even-strip cols 0:256]
            masks_np = np.ascontiguousarray(
                np.concatenate([m1[:, 256:512], m0[:, 0:256]], axis=1)
            )
            in_maps.append(
                {
                    "xt": xt_in,
                    "wq": wq_pack,
                    "wkv": wkv_pack,
                    "bias_q": bias_q,
                    "bias_kv": bias_kv,
                    "masks": masks_np,
                }
            )
    return in_maps


def _combine(results):
    out = np.empty((B, T, H), np.float32)
    p1 = _perm(1)
    for b in range(B):
        a0 = results[2 * b]["out"].astype(np.float64)
        a1 = results[2 * b + 1]["out"].astype(np.float64)
        a1 = a1[:, p1]  # un-rotate core-1 columns (involutive perm)
        num = a0[:H] + a1[:H]
        den = a0[H] + a1[H]
        out[b] = (num / den).T.astype(np.float32)
    return out


def _run(trace=False, **inputs):
    from concourse import bass_utils

    nc = _build()
    in_maps = _make_in_maps(
        np.asarray(inputs["x"], np.float32),
        np.asarray(inputs["Wq"], np.float32),
        np.asarray(inputs["bq"], np.float32),
        np.asarray(inputs["Wk"], np.float32),
        np.asarray(inputs["bk"], np.float32),
        np.asarray(inputs["Wv"], np.float32),
        np.asarray(inputs["bv"], np.float32),
    )
    res = bass_utils.run_bass_kernel_spmd(
        nc, in_maps, list(range(NCORES)), trace=trace
    )
    return _combine(res.results), res.exec_time_ns


def kernel(**inputs):
    out, _ = _run(trace=False, **inputs)
    return out


# revision 10
# speedup vs baseline: 1.1170x; 1.0433x over previous
"""Trainium2 Bass kernel: single-head causal attention.

B=4, T=4096, E=512, H=64, fp32 in/out.

Sharding: 2 cores per batch sample. Each core computes partial softmax
(numerator and denominator) for ALL 4096 queries of its sample over HALF
the keys: core 2b takes even 128-key-strips, core 2b+1 odd strips. The
host combines partials: out = (num0+num1)/(den0+den1).

Token rotation: the host rotates every 256-token block by 128*rho so
each core's keys are the first 128 tokens of every 256-block (identical
SPMD program on all cores). Host un-permutes output columns; the causal
masks carry the rotation.

Device kernel per core (bf16 matmul operands, fp32 PSUM accumulate):
  - ROW-TILED scores: the two strips of a pair run concurrently in the
    PE array (strip A rows 0:64, strip B rows 64:128 via operand base
    partitions). Q projection duplicated to both partition halves
    ([Wq|Wq] stationary). The kv projection is PARITY-SPLIT so odd
    strips' K^T lands directly at partitions 64:128: even-strip token
    columns use the [Wk|Wv] stationary, odd-strip columns [Wv|Wk]
    (so odd strips' V^T is at rows 0:64 instead).
  - V^T -> V (natural [k,h]) via PE transpose (4 per kv chunk into one
    bf16 PSUM tile) + one strided DVE copy per chunk; a ones column is
    packed after each V strip so the PV matmul (M=65) also produces the
    softmax denominator (partition-dim reduction on the PE).
  - exp on the scalar engine with fused 1/sqrt(H) scale; no max
    subtraction (scores bounded; fp32 exp cannot overflow here).
  - Diagonal pair of each chunk: the fully-masked half of the odd strip
    is skipped (scores/exp/PV restricted to query cols 256:512); the
    exp covers the contiguous [256:1024] window of the pair tile.
  - Causal mask applied multiplicatively after exp on the vector
    engine (256-col windows per diagonal strip).
  - Emission is software-pipelined: chunk c's first scores pair goes
    out before the c+1 projections, and scores/exp of pair j+1 precede
    PV of pair j, so the in-order PE never waits on exp or on input
    DMAs.
  - PE warm-up matmuls run during the input DMA window so real matmuls
    start at the full 2.4 GHz clock (HAM un-throttled).
"""

import functools

import numpy as np
import ml_dtypes

B, T, E, H = 4, 4096, 512, 64
NCORES = 8
NCHUNK = 8  # 512-query chunks per sample
CHUNK = T // NCHUNK  # 512
NSTRIP = 16  # local 128-key strips per core (half of T/128)
VSTRIDE = 80  # per-strip stride in the packed V tile

bf16 = ml_dtypes.bfloat16


@functools.lru_cache(maxsize=1)
def _build():
    import concourse.mybir as mybir
    from concourse import bacc
    from concourse.masks import make_identity
    import concourse.tile as tile

    dt_bf = mybir.dt.bfloat16
    dt_f32 = mybir.dt.float32

    nc = bacc.Bacc("TRN2", target_bir_lowering=False, num_devices=NCORES)

    # x^T, rotated, (quarter, e-strip)-blocked:
    # [4 quarters, 128, 4 e-strips, 1024 tokens]
    xt = nc.dram_tensor("xt", [4, 128, 4, T // 4], dt_bf, kind="ExternalInput")
    # [Wq | Wq] per e-strip (duplicated for row-tiled scores)
    wq = nc.dram_tensor("wq", [128, 4 * 128], dt_bf, kind="ExternalInput")
    # per e-strip: [Wk|Wv] (even-strip cols) then [Wv|Wk] (odd-strip cols)
    wkv = nc.dram_tensor("wkv", [128, 4 * 256], dt_bf, kind="ExternalInput")
    bias_q = nc.dram_tensor("bias_q", [128, 1], dt_f32, kind="ExternalInput")
    # [bk|bv] and [bv|bk]
    bias_kv = nc.dram_tensor("bias_kv", [128, 2], dt_f32, kind="ExternalInput")
    masks = nc.dram_tensor("masks", [128, CHUNK], dt_bf, kind="ExternalInput")
    out_d = nc.dram_tensor("out", [H + 1, T], dt_f32, kind="ExternalOutput")

    AF = mybir.ActivationFunctionType

    with tile.TileContext(nc) as tc:
        with (
            tc.tile_pool(name="const", bufs=1) as cpool,
            tc.tile_pool(name="xt_pool", bufs=1) as xpool,
            tc.tile_pool(name="q_pool", bufs=NCHUNK) as qpool,
            tc.tile_pool(name="kv_pool", bufs=4) as kvpool,
            tc.tile_pool(name="v_pool", bufs=1) as vpool,
            tc.tile_pool(name="p_pool", bufs=3) as ppool,
            tc.tile_pool(name="o_pool", bufs=2) as opool,
            tc.tile_pool(name="ps_proj", bufs=2, space="PSUM") as pspr_pool,
            tc.tile_pool(name="ps_s", bufs=2, space="PSUM") as pss_pool,
            tc.tile_pool(name="ps_o", bufs=2, space="PSUM") as pso_pool,
        ):
            # ---- input DMAs. The critical path (wkv + quarter-0
            # e-strips) leads on the sync queue, followed by the other
            # quarters in order; the scalar queue carries only the small
            # weight/bias/mask transfers so nothing competes with the
            # quarter-0 stream for SDMA bandwidth ----
            xt_sb = xpool.tile([128, 4 * T], dt_bf)
            wkv_sb = cpool.tile([128, 4 * 256], dt_bf)
            nc.sync.dma_start(wkv_sb, wkv.ap())
            for es in range(4):
                nc.sync.dma_start(
                    xt_sb[:, es * 1024 : (es + 1) * 1024], xt.ap()[0][:, es, :]
                )
            wq_sb = cpool.tile([128, 4 * 128], dt_bf)
            nc.scalar.dma_start(wq_sb, wq.ap())
            bkv_sb = cpool.tile([128, 2], dt_f32)
            nc.scalar.dma_start(bkv_sb, bias_kv.ap())
            bq_sb = cpool.tile([128, 1], dt_f32)
            nc.scalar.dma_start(bq_sb, bias_q.ap())
            masks_sb = cpool.tile([128, CHUNK], dt_bf)
            nc.scalar.dma_start(masks_sb, masks.ap())
            for qd in range(1, 4):
                nc.sync.dma_start(
                    xt_sb[:, qd * T : qd * T + T // 2],
                    xt.ap()[qd][:, 0:2, :].rearrange("p a t -> p (a t)"),
                )
                nc.sync.dma_start(
                    xt_sb[:, qd * T + T // 2 : (qd + 1) * T],
                    xt.ap()[qd][:, 2:4, :].rearrange("p a t -> p (a t)"),
                )
            ident = cpool.tile([128, 128], dt_bf)
            make_identity(nc, ident)
            junk_sb = cpool.tile([128, CHUNK], dt_bf)
            nc.vector.memset(junk_sb, 0.0)

            # ---- PE warm-up (results never read) ----
            for i in range(2):
                ps_w = pss_pool.tile([128, 2 * CHUNK], dt_f32, tag="pss")
                for r in range(2 if i == 0 else 1):
                    nc.tensor.matmul(
                        ps_w[:, r * CHUNK : (r + 1) * CHUNK],
                        lhsT=junk_sb[:, 0:128],
                        rhs=junk_sb,
                        start=True,
                        stop=True,
                        skip_group_check=True,
                    )

            # packed V (natural [k,h] layout + ones column for denominator)
            v_nat = vpool.tile([128, NSTRIP * VSTRIDE], dt_bf)
            v3 = v_nat.rearrange("p (s c) -> p s c", c=VSTRIDE)
            nc.vector.memset(v3[:, :, 64:65], 1.0)

            def xt_block(qd, es):
                off = (qd * 4 + es) * 1024
                return xt_sb[:, off : off + 1024]

            scale = 1.0 / float(np.sqrt(H))
            kv_tiles = []
            q_tiles = []

            def kv_proj(ckv):
                # kv tile layout: cols [0:256] = even strips (0,2) with
                # [K^T;V^T] rows, cols [256:512] = odd strips (1,3) with
                # [V^T;K^T] rows (so odd K^T sits at partitions 64:128).
                # The two parity accumulation chains must live in
                # DIFFERENT PSUM banks: a start=True matmul clears the
                # has_written bits of its whole bank, so a second chain's
                # start would break the first chain's accumulation.
                ps_e = pspr_pool.tile([128, 256], dt_f32, tag="proj")
                ps_od = pss_pool.tile([128, 2 * CHUNK], dt_f32, tag="pss")
                for es in range(4):
                    key_rhs = xt_block(ckv, es).rearrange(
                        "p (a two b) -> p a two b", two=2, b=128
                    )[:, :, 0, :]
                    nc.tensor.matmul(
                        ps_e,
                        lhsT=wkv_sb[:, es * 256 : es * 256 + 128],
                        rhs=key_rhs[:, 0::2, :],
                        start=(es == 0),
                        stop=(es == 3),
                    )
                    nc.tensor.matmul(
                        ps_od[:, 0:256],
                        lhsT=wkv_sb[:, es * 256 + 128 : es * 256 + 256],
                        rhs=key_rhs[:, 1::2, :],
                        start=(es == 0),
                        stop=(es == 3),
                    )
                kv_sb = kvpool.tile([128, CHUNK], dt_bf, tag="kv")
                nc.vector.tensor_scalar_add(
                    kv_sb[:, 0:256], ps_e, bkv_sb[:, 0:1]
                )
                nc.vector.tensor_scalar_add(
                    kv_sb[:, 256:512], ps_od[:, 0:256], bkv_sb[:, 1:2]
                )
                kv_tiles.append(kv_sb)

            def v_transpose(ckv):
                # V^T -> natural V strips via PE transpose. Even strips'
                # V^T is at rows 64:128 (-> V cols 64:128 of the
                # transpose), odd strips' at rows 0:64 (-> V cols 0:64).
                # Transposed block order: [s0, s2, s1, s3].
                kv_sb = kv_tiles[ckv]
                ps_tr = pspr_pool.tile([128, CHUNK], dt_bf, tag="proj")
                for j in range(4):
                    nc.tensor.transpose(
                        ps_tr[:, j * 128 : (j + 1) * 128],
                        kv_sb[:, j * 128 : (j + 1) * 128],
                        ident,
                    )
                sl = v3[:, 4 * ckv : 4 * ckv + 4, 0:64]
                ev = ps_tr[:, 64:320].rearrange("p (s c) -> p s c", c=128)
                od = ps_tr[:, 256:512].rearrange("p (s c) -> p s c", c=128)
                # even strips s0, s2 -> v slots 0, 2; odd s1, s3 -> 1, 3
                nc.vector.tensor_copy(sl[:, 0::2, :], ev[:, :, 0:64])
                nc.vector.tensor_copy(sl[:, 1::2, :], od[:, :, 0:64])

            def q_proj(c):
                ps_q = pspr_pool.tile([128, CHUNK], dt_f32, tag="proj")
                for es in range(4):
                    nc.tensor.matmul(
                        ps_q,
                        lhsT=wq_sb[:, es * 128 : (es + 1) * 128],
                        rhs=xt_block(c // 2, es)[
                            :, (c % 2) * CHUNK : (c % 2) * CHUNK + CHUNK
                        ],
                        start=(es == 0),
                        stop=(es == 3),
                    )
                q_sb = qpool.tile([128, CHUNK], dt_bf, tag="q")
                nc.vector.tensor_scalar_add(q_sb, ps_q, bq_sb)
                q_tiles.append(q_sb)

            def k_lo(l):
                # even strip l: K^T at partitions 0:64, even half cols
                o = ((l % 4) // 2) * 128
                return kv_tiles[l // 4][0:64, o : o + 128]

            def k_hi(l):
                # odd strip l: K^T at partitions 64:128, odd half cols
                o = 256 + ((l % 4) // 2) * 128
                return kv_tiles[l // 4][64:128, o : o + 128]

            kv_proj(0)
            q_proj(0)
            for c in range(NCHUNK):
                ns = 2 * (c + 1)
                q_sb = q_tiles[c]
                ps_o = pso_pool.tile([H + 1, CHUNK], dt_f32, tag="pso")

                def scores_exp(j):
                    le, lo = 2 * j, 2 * j + 1
                    diag = j == c
                    ps_s = pss_pool.tile([128, 2 * CHUNK], dt_f32, tag="pss")
                    p_sb = ppool.tile([128, 2 * CHUNK], dt_bf, tag="p")
                    if not diag:
                        nc.tensor.matmul(
                            ps_s[:, 0:CHUNK],
                            lhsT=k_lo(le),
                            rhs=q_sb[0:64, :],
                            start=True,
                            stop=True,
                        )
                        nc.tensor.matmul(
                            ps_s[:, CHUNK : 2 * CHUNK],
                            lhsT=k_hi(lo),
                            rhs=q_sb[64:128, :],
                            start=True,
                            stop=True,
                        )
                        nc.scalar.activation(p_sb, ps_s, AF.Exp, scale=scale)
                    else:
                        # diagonal pair: odd strip only needs query cols
                        # 256:512; layout [odd | even] so one exp covers
                        # the contiguous [256:1024] window
                        nc.tensor.matmul(
                            ps_s[:, 256:CHUNK],
                            lhsT=k_hi(lo),
                            rhs=q_sb[64:128, 256:CHUNK],
                            start=True,
                            stop=True,
                        )
                        nc.tensor.matmul(
                            ps_s[:, CHUNK : 2 * CHUNK],
                            lhsT=k_lo(le),
                            rhs=q_sb[0:64, :],
                            start=True,
                            stop=True,
                        )
                        nc.scalar.activation(
                            p_sb[:, 256 : 2 * CHUNK],
                            ps_s[:, 256 : 2 * CHUNK],
                            AF.Exp,
                            scale=scale,
                        )
                        nc.vector.tensor_mul(
                            p_sb[:, 256:CHUNK],
                            p_sb[:, 256:CHUNK],
                            masks_sb[:, 0:256],
                        )
                        nc.vector.tensor_mul(
                            p_sb[:, CHUNK : CHUNK + 256],
                            p_sb[:, CHUNK : CHUNK + 256],
                            masks_sb[:, 256:512],
                        )
                    return p_sb

                def pv(j, p_sb):
                    le, lo = 2 * j, 2 * j + 1
                    diag = j == c
                    if not diag:
                        nc.tensor.matmul(
                            ps_o,
                            lhsT=v_nat[:, le * VSTRIDE : le * VSTRIDE + 65],
                            rhs=p_sb[:, 0:CHUNK],
                            start=(le == 0),
                            stop=False,
                        )
                        nc.tensor.matmul(
                            ps_o,
                            lhsT=v_nat[:, lo * VSTRIDE : lo * VSTRIDE + 65],
                            rhs=p_sb[:, CHUNK : 2 * CHUNK],
                            start=False,
                            stop=(lo == ns - 1),
                        )
                    else:
                        nc.tensor.matmul(
                            ps_o,
                            lhsT=v_nat[:, le * VSTRIDE : le * VSTRIDE + 65],
                            rhs=p_sb[:, CHUNK : 2 * CHUNK],
                            start=(le == 0),
                            stop=False,
                        )
                        nc.tensor.matmul(
                            ps_o[:, 256:CHUNK],
                            lhsT=v_nat[:, lo * VSTRIDE : lo * VSTRIDE + 65],
                            rhs=p_sb[:, 256:CHUNK],
                            start=False,
                            stop=(lo == ns - 1),
                            skip_group_check=True,
                        )

                # chunk c's first scores pair goes out BEFORE the c+1
                # projections (which may wait on later input DMAs)
                prev = scores_exp(0)
                if c % 2 == 0:
                    v_transpose(c // 2)
                if c + 1 < NCHUNK:
                    if (c + 1) % 2 == 0:
                        kv_proj((c + 1) // 2)
                    q_proj(c + 1)
                for j in range(1, c + 1):
                    cur = scores_exp(j)
                    pv(j - 1, prev)
                    prev = cur
                pv(c, prev)

                o_sb = opool.tile([H + 1, CHUNK], dt_f32, tag="o")
                nc.vector.tensor_copy(o_sb, ps_o)
                nc.sync.dma_start(
                    out_d.ap()[:, c * CHUNK : (c + 1) * CHUNK], o_sb
                )

    nc.compile()
    return nc


def _perm(rho):
    """Rotated-order permutation: rotated position i holds original token
    perm[i]. Involutive (half swap within each 256-block)."""
    i = np.arange(T)
    return (i // 256) * 256 + ((i % 256) + 128 * rho) % 256


def _make_in_maps(x, Wq, bq, Wk, bk, Wv, bv):
    wq_r = Wq.reshape(4, 128, 64)
    wq_pack = np.ascontiguousarray(
        np.concatenate([wq_r, wq_r], axis=2).transpose(1, 0, 2).reshape(128, 512)
    ).astype(bf16)
    wk_r = Wk.reshape(4, 128, 64)
    wv_r = Wv.reshape(4, 128, 64)
    # per e-strip: [Wk|Wv] then [Wv|Wk]
    wkv_pack = np.ascontiguousarray(
        np.concatenate([wk_r, wv_r, wv_r, wk_r], axis=2)
        .transpose(1, 0, 2)
        .reshape(128, 1024)
    ).astype(bf16)
    bias_q = np.ascontiguousarray(
        np.concatenate([bq, bq])[:, None]
    ).astype(np.float32)
    bias_kv = np.ascontiguousarray(
        np.stack([np.concatenate([bk, bv]), np.concatenate([bv, bk])], axis=1)
    ).astype(np.float32)

    kk = np.arange(128)[:, None]
    in_maps = []
    for b in range(B):
        xt_b = np.ascontiguousarray(x[b].T).astype(bf16).reshape(4, 128, T)
        for rho in range(2):
            perm = _perm(rho)
            xt_rot = xt_b[:, :, perm]  # rotated token order
            xt_in = np.ascontiguousarray(
                xt_rot.reshape(4, 128, 4, T // 4).transpose(2, 1, 0, 3)
            )
            v = perm[:CHUNK]
            m0 = (kk - v[None, :] <= -128 * rho).astype(bf16)
            m1 = (kk - v[None, :] <= -256 - 128 * rho).astype(bf16)
            # [odd-strip window cols 256:512 | even-strip cols 0:256]
            masks_np = np.ascontiguousarray(
                np.concatenate([m1[:, 256:512], m0[:, 0:256]], axis=1)
            )
            in_maps.append(
                {
                    "xt": xt_in,
                    "wq": wq_pack,
                    "wkv": wkv_pack,
                    "bias_q": bias_q,
                    "bias_kv": bias_kv,
                    "masks": masks_np,
                }
            )
    return in_maps


def _combine(results):
    out = np.empty((B, T, H), np.float32)
    p1 = _perm(1)
    for b in range(B):
        a0 = results[2 * b]["out"].astype(np.float64)
        a1 = results[2 * b + 1]["out"].astype(np.float64)
        a1 = a1[:, p1]  # un-rotate core-1 columns (involutive perm)
        num = a0[:H] + a1[:H]
        den = a0[H] + a1[H]
        out[b] = (num / den).T.astype(np.float32)
    return out


def _run(trace=False, **inputs):
    from concourse import bass_utils

    nc = _build()
    in_maps = _make_in_maps(
        np.asarray(inputs["x"], np.float32),
        np.asarray(inputs["Wq"], np.float32),
        np.asarray(inputs["bq"], np.float32),
        np.asarray(inputs["Wk"], np.float32),
        np.asarray(inputs["bk"], np.float32),
        np.asarray(inputs["Wv"], np.float32),
        np.asarray(inputs["bv"], np.float32),
    )
    res = bass_utils.run_bass_kernel_spmd(
        nc, in_maps, list(range(NCORES)), trace=trace
    )
    return _combine(res.results), res.exec_time_ns


def kernel(**inputs):
    out, _ = _run(trace=False, **inputs)
    return out
